# revision 2
# baseline (speedup 1.0000x reference)
"""Grouped-dequant GEMM (y = x @ (W * group_scales)^T + bias) on 8 TRN2 NeuronCores.

2D tensor-parallel sharding: 2-way along tokens (M) x 4-way along out_features.
Each core computes a [4096 x 1024] output block (same per-core FLOPs as pure
column-parallel, matmul count unchanged), but:
  - per-core x DMA traffic halves (33.5 MB vs 67 MB) -> the early phase is no
    longer HBM-supply-bound and chip-wide HBM load drops ~30%;
  - the first matmul's weight-tile payload halves (w+s k0 chunk = 1 MB vs 2 MB)
    -> shorter head.
All FLOPs (dequant multiply, GEMM, bias add) run on device; the host only does
sharding + layout transforms.

Self-contained: hardcodes shapes from the problem spec.
  x      (4, 2048, 4096) fp16
  weight (4096, 4096)    fp16
  scales (4096, 32)      fp16   group size g=128 along in_features
  bias   (4096,)         fp16
  types  (64, 32)        int32  (unused by the exact-dequant reference math)
"""

import sys
import types as _types

sys.path.insert(0, "/opt/trn_rl_repo")


def _install_ntff_hook_shim():
    """antenv.axon_hooks is missing in this image; register the NTFF profile
    hook from trn_agent_boot so run_bass_kernel_spmd(trace=True) works."""
    if "antenv.axon_hooks" in sys.modules:
        return
    mod = _types.ModuleType("antenv.axon_hooks")
    try:
        import trn_agent_boot.trn_boot as tb

        hook = tb._ntff_profile_via_ctypes("/opt/axon/libaxon_pjrt.so")
    except Exception:
        hook = None
    mod.get_axon_ntff_profile_hook = lambda: hook
    mod.set_axon_ntff_profile_hook = lambda h: None
    sys.modules["antenv.axon_hooks"] = mod


_install_ntff_hook_shim()

import numpy as np

import concourse.bacc as bacc
import concourse.mybir as mybir
import concourse.tile as tile
from concourse.bass import ds, ts
from concourse.bass_utils import run_bass_kernel_spmd
from concourse.bass import _add_dep_helper
from concourse.kernels.tile_matmul import (
    ShapeInfo,
    composable_matmul_tile_kernel,
)

B, S, I, O, G = 4, 2048, 4096, 4096, 128
N_CORES = 8
M_WAYS, O_WAYS = 2, 4
MC = (B * S) // M_WAYS  # 4096 tokens per core
OC = O // O_WAYS  # 1024 output features per core
P = 128
KT, MT, KS = I // 512, MC // 512, 4  # 8 k-tiles, 8 m-tiles, 4 k-subtiles
NT = OC // 512  # 2 n-chunks of 512

_cached_nc = None


def _build_bass():
    """Build + compile the per-core Bass program (same graph on all 8 cores).

    Computes y = xT.T @ w_deqT + bias where
      kxm = xT   [I, MC]  (streamed; stationary operand of the matmuls)
      kxn = wT   [I, OC]  (dequantized in SBUF on load, then resident)
      out = y    [MC, OC]
    """
    global _cached_nc
    if _cached_nc is not None:
        return _cached_nc

    nc = bacc.Bacc(
        "TRN2", target_bir_lowering=False, debug=False, num_devices=N_CORES
    )
    f16, f32 = mybir.dt.float16, mybir.dt.float32

    # Inputs are pre-permuted on the host into tile-major layouts so every
    # SBUF tile's per-partition data is CONTIGUOUS in DRAM (4 KiB bursts).
    xH = nc.dram_tensor("xH", [KT, MT, P, KS, 512], f16, kind="ExternalInput").ap()
    wH = nc.dram_tensor("wH", [KT, NT, P, KS, 512], f16, kind="ExternalInput").ap()
    sH = nc.dram_tensor("sH", [KT, NT, P, KS, 512], f16, kind="ExternalInput").ap()
    bias_rep = nc.dram_tensor("bias_rep", [P, OC], f32, kind="ExternalInput").ap()
    # Output is tile-major too: yH[mt, nt, pi, po, o] = y[mt*512 + po*128 + pi,
    # nt*512 + o] (4 KiB contiguous per partition per store; host un-permutes).
    yH = nc.dram_tensor("yH", [MT, NT, P, 4, 512], f16, kind="ExternalOutput").ap()

    with tile.TileContext(nc) as tc:
        from contextlib import ExitStack

        with ExitStack() as ctx:
            kxm_pool = ctx.enter_context(tc.tile_pool(name="kxm_pool", bufs=20))
            # 16 permanent dequantized-weight tiles (one per (k_tile, n_chunk)):
            # produced once, reused across all m-tiles and snake n-passes.
            kxn_pool = ctx.enter_context(tc.tile_pool(name="kxn_pool", bufs=16))
            wstage_pool = ctx.enter_context(tc.tile_pool(name="wstage", bufs=2))
            sdeq_pool = ctx.enter_context(tc.tile_pool(name="sdeq", bufs=3))
            const_pool = ctx.enter_context(tc.tile_pool(name="const", bufs=1))

            # Weight-side/epilogue DMAs ride the Scalar/GpSimd queues so the x
            # stream owns the Sync HWDGE ring outright; descriptor generation
            # (DIRECT2D, ~0.7us per dma_start) also stays off Sync's sequencer.
            bias_sb = const_pool.tile([P, OC], f32)
            nc.scalar.dma_start(bias_sb[:], bias_rep[:, :])

            kxm_shape = ShapeInfo(pdims=((P, I // P),), fdims=(MC,))
            kxn_shape = ShapeInfo(pdims=((P, I // P),), fdims=(OC,))

            s0_dma = []
            wdeq_cache = {}

            def kxn_producer(nc, md):
                # The snake order re-requests (k, n) tiles on every n-switch;
                # dequantized tiles are cached in SBUF permanently, so each of
                # the 16 (k, n) combos is loaded + dequantized exactly once:
                # w_deq[i, o] = w[i, o] * scales[o, i // G].
                key = (md.k_tile_idx, md.n_tile_idx)
                if key in wdeq_cache:
                    return wdeq_cache[key]
                t = kxn_pool.tile([P, md.k_subtiles, md.n_tile], f16, tag="wdeq")
                w = wstage_pool.tile([P, md.k_subtiles, md.n_tile], f16)
                s = sdeq_pool.tile([P, md.k_subtiles, md.n_tile], f16)
                nc.scalar.dma_start(w[:], wH[md.k_tile_idx, md.n_tile_idx])
                si = nc.gpsimd.dma_start(s[:], sH[md.k_tile_idx, md.n_tile_idx])
                if key == (0, 0) and not s0_dma:
                    s0_dma.append(si.ins)
                # Dequantize per-subtile so the first matmuls only wait on
                # subtile 0's multiply.
                for ks in range(md.k_subtiles):
                    nc.vector.tensor_mul(t[:, ks, :], w[:, ks, :], s[:, ks, :])
                wdeq_cache[key] = t
                return t

            def kxm_producer(nc, md):
                assert md.k_subtiles == KS and md.m_tile == 512
                t = kxm_pool.tile([P, md.k_subtiles, md.m_tile], f16, tag="kxm")
                di = nc.sync.dma_start(t[:], xH[md.k_tile_idx, md.m_tile_idx])
                if md.k_tile_idx == 1 and md.m_tile_idx == 0 and s0_dma:
                    # Let k0's w/scales transfers finish before the x
                    # prefetch flood claims HBM, so the first matmul's
                    # dequant chain completes ASAP.
                    _add_dep_helper(
                        di.ins,
                        s0_dma[0],
                        sync=True,
                        reason="x prefetch yields HBM to k0 dequant inputs",
                    )
                return t

            def bias_reducer(nc, psum, sbuf, md):
                # sbuf(fp16) = psum(fp32) + bias(fp32), fused cast on DVE.
                n0 = md.n_tile_idx * md.n_tile + md.n_subtile_idx * md.n_subtile
                nc.vector.tensor_tensor(
                    sbuf,
                    psum,
                    bias_sb[:, ds(n0, md.n_subtile_slice_size)],
                    mybir.AluOpType.add,
                )

            def mxn_consumer(nc, mxn_tile, md):
                assert md.m_subtiles == 4 and md.n_tile_idx in (0, 1)
                nc.scalar.dma_start(
                    yH[md.m_tile_idx, md.n_tile_idx], mxn_tile[:, :, :]
                )

            composable_matmul_tile_kernel(
                tc=tc,
                kxm_shape=kxm_shape,
                kxn_shape=kxn_shape,
                output_type=mybir.dt.float16,
                kxm_producer=kxm_producer,
                kxn_producer=kxn_producer,
                mxn_consumer=mxn_consumer,
                mxn_subtile_reducer=bias_reducer,
                psum_n_bufs=2,
                cache_tiles=True,
            )

    nc.compile()
    _cached_nc = nc
    return nc


def kernel(x, weight, scales, bias, types, g, _want_exec_time=False):
    assert int(g) == G
    x = np.asarray(x)
    weight = np.asarray(weight)
    scales = np.asarray(scales)
    bias = np.asarray(bias)
    assert x.shape == (B, S, I) and weight.shape == (O, I)

    nc = _build_bass()

    # Host-side layout: tile-major permutations + per-core shards (no math
    # here).  Index maps (for m-group mg, o-chunk oc):
    #   xH[kt, mt, pi, ks, m] = x[mg*4096 + mt*512 + m, kt*512 + ks*128 + pi]
    #   wH[kt, nt, pi, ks, o] = weight[oc*1024 + nt*512 + o, kt*512 + ks*128 + pi]
    #   sH[kt, nt, pi, ks, o] = scales[oc*1024 + nt*512 + o, (kt*512 + ks*128) // G]
    M = B * S
    x2 = x.reshape(M, I)
    xHs = [
        np.ascontiguousarray(
            x2[mg * MC : (mg + 1) * MC]
            .reshape(MT, 512, KT, KS, P)
            .transpose(2, 0, 4, 3, 1)
        )
        for mg in range(M_WAYS)
    ]
    wT = weight.T  # [I, O]
    srT = np.repeat(scales, G, axis=1).T  # [I, O]
    bias_rep = np.broadcast_to(bias.astype(np.float32)[None, :], (P, O))

    in_maps = []
    for c in range(N_CORES):
        mg, oc = c // O_WAYS, c % O_WAYS
        sl = slice(oc * OC, (oc + 1) * OC)
        in_maps.append(
            {
                "xH": xHs[mg],
                "wH": np.ascontiguousarray(
                    wT[:, sl].reshape(KT, KS, P, NT, 512).transpose(0, 3, 2, 1, 4)
                ),
                "sH": np.ascontiguousarray(
                    srT[:, sl].reshape(KT, KS, P, NT, 512).transpose(0, 3, 2, 1, 4)
                ),
                "bias_rep": np.ascontiguousarray(bias_rep[:, sl]),
            }
        )

    res = run_bass_kernel_spmd(
        nc, in_maps, core_ids=list(range(N_CORES)), trace=_want_exec_time
    )

    y = np.empty((M, O), dtype=np.float16)
    for c in range(N_CORES):
        mg, oc = c // O_WAYS, c % O_WAYS
        yHc = res.results[c]["yH"]  # [MT, NT, P, 4, 512] tile-major
        y[mg * MC : (mg + 1) * MC, oc * OC : (oc + 1) * OC] = (
            yHc.transpose(0, 3, 2, 1, 4).reshape(MC, OC)
        )
    out = y.reshape(B, S, O)
    if _want_exec_time:
        return out, res.exec_time_ns
    return out
